# revision 1
# baseline (speedup 1.0000x reference)
"""NomicBertAttention on 8 Trainium2 NeuronCores.

Sharding: 8-way head tensor-parallelism (2 heads/core, both batches).
Per 1024-column window of the flattened (b,s) axis, an fp16 AllToAll
re-shards ctx^T by sequence rows (each core owns one 128-row block per
window), and the row-parallel out-proj + residual + LayerNorm for that
window is software-pipelined behind the next window's attention.

Attention matmuls run in fp8e4m3 DoubleRow mode (0.5 cycles/row, fp32
PSUM): projections contract K=256 per weight load, scores contract the
64-dim head as 2x32 (head-dim halves interleaved in the free dim via a
host-side feature permutation), and ctx contracts 2 t-chunks (K=2x128)
with a ones-column in V producing the softmax denominator. Scale
folding keeps fp8 in its normal range: W_{q,k,v} are scaled x16
host-side, the exp activation applies 1/(64*256) (RoPE q,k are x16 and
the reference's double 1/8 scaling), and Wo carries the remaining 1/16.
The residual/LayerNorm path stays fp32, so fp8 noise only touches the
small attention contribution. RoPE rotate-half rides a constant +-1
matrix on the PE; LayerNorm stats use bn_stats/bn_aggr and a
magic-constant rsqrt on the DVE so the ACT engine stays on the Exp
table (the last window uses ACT Sqrt since no exp follows it).
"""

import numpy as np
import ml_dtypes
import concourse.bacc as bacc
import concourse.mybir as mybir
import concourse.tile as tile
from concourse.bass_utils import run_bass_kernel_spmd
from concourse.masks import make_identity

F32 = mybir.dt.float32
F16 = mybir.dt.float16
F8 = mybir.dt.float8e4
I32 = mybir.dt.int32
DR = mybir.MatmulPerfMode.DoubleRow
MULT = mybir.AluOpType.mult
ADD = mybir.AluOpType.add
SUB = mybir.AluOpType.subtract
XOR = mybir.AluOpType.bitwise_xor
SHR = mybir.AluOpType.arith_shift_right
BYPASS = mybir.AluOpType.bypass
EXP = mybir.ActivationFunctionType.Exp
IDENT = mybir.ActivationFunctionType.Identity
SQRT = mybir.ActivationFunctionType.Sqrt

B, S, D, H, HD = 2, 2048, 1024, 16, 64
NC = 8
HPC = H // NC          # 2 heads per core
F = HPC * HD           # 128 projected features per core
NSEQ = B * S           # 4096 flattened rows
ROWS = NSEQ // NC      # 512 output rows per core (4 blocks of 128)
NW = 4                 # 1024-column windows
TB = S // 128          # 16 t-chunks per batch
NPAIR = TB // 2        # 8 t-chunk pairs per batch
MV = 80                # ctx stationary free (64 v + 1 ones + 15 pad; mult of 16)
WSCALE = 16.0          # fp8 range scaling folded into Wq/Wk/Wv
EXP_SCALE = 1.0 / (64.0 * WSCALE * WSCALE)
EPS = 1e-12
RSQRT_MAGIC = 0x5F3759DF

LAST_RESULTS = None


def _build():
    nc = bacc.Bacc("TRN2", target_bir_lowering=False, debug=False, num_devices=NC)

    xT = nc.dram_tensor("xT", [D, NSEQ], F8, kind="ExternalInput")
    wq = nc.dram_tensor("wq", [128, D], F8, kind="ExternalInput")
    wk = nc.dram_tensor("wk", [128, D], F8, kind="ExternalInput")
    wv = nc.dram_tensor("wv", [128, D], F8, kind="ExternalInput")
    bqd = nc.dram_tensor("bq", [F, 1], F32, kind="ExternalInput")
    bkd = nc.dram_tensor("bk", [F, 1], F32, kind="ExternalInput")
    bvd = nc.dram_tensor("bv", [F, 1], F32, kind="ExternalInput")
    wo8d = nc.dram_tensor("wo8", [128, 2 * 4 * 2 * 512], F8, kind="ExternalInput")
    cs2d = nc.dram_tensor("cs2", [128, S], F16, kind="ExternalInput")
    sn2d = nc.dram_tensor("sn2", [128, S], F16, kind="ExternalInput")
    residd = nc.dram_tensor("resid", [ROWS, D], F32, kind="ExternalInput")
    lnwd = nc.dram_tensor("lnw", [128, D], F32, kind="ExternalInput")
    lnbd = nc.dram_tensor("lnb", [128, D], F32, kind="ExternalInput")
    outd = nc.dram_tensor("out", [ROWS, D], F32, kind="ExternalOutput")
    import os as _os0
    _dump = bool(_os0.environ.get("KD_DUMP"))
    if _dump:
        cto = nc.dram_tensor("cto", [NW, 128, 1024], F16, kind="ExternalOutput")
        cfo = nc.dram_tensor("cfo", [NW, 128, 1024], F16, kind="ExternalOutput")
        oso = nc.dram_tensor("oso", [NW, 128, D], F32, kind="ExternalOutput")

    with tile.TileContext(nc) as tc:
        with (
            tc.tile_pool(name="qk", bufs=1) as qkpool,
            tc.tile_pool(name="wpool", bufs=1) as wpool,
            tc.tile_pool(name="xpool", bufs=2) as xpool,
            tc.tile_pool(name="rope", bufs=2) as rpool,
            tc.tile_pool(name="exps", bufs=3) as epool,
            tc.tile_pool(name="ctxp", bufs=2) as ctxpool,
            tc.tile_pool(name="bcastp", bufs=2) as bpool,
            tc.tile_pool(name="small", bufs=4) as spool,
            tc.tile_pool(name="tail", bufs=2) as lpool,
            tc.tile_pool(name="psS", bufs=2, space="PSUM") as psS,
            tc.tile_pool(name="psC", bufs=4, space="PSUM") as psC,
            tc.tile_pool(name="dram", bufs=1, space="DRAM") as dpool,
        ):
            # ---- resident tensors
            # q/k RoPE'd fp8: [32h+dlo (64 parts), d-half, pos]
            qS8 = qkpool.tile([64, 2, NSEQ], F8, tag="qS8")
            kS8 = qkpool.tile([64, 2, NSEQ], F8, tag="kS8")
            # v natural fp8 + ones col (=4, folds a 1/4 into the denominator
            # so ctile lands at 4*ctx, matching the x4-scaled fp8 Wo)
            vaug = qkpool.tile([128, 2 * NPAIR, 2, HPC, MV], F8, tag="vaug")
            nc.vector.memset(vaug[:, :, :, :, HD : HD + 1], 4.0)
            nc.vector.memset(vaug[:, :, :, :, HD + 1 : MV], 0.0)

            a2a_in = {}
            a2a_out = {}
            for w in range(NW):
                a2a_in[w] = dpool.tile([NC, 128, 128], F8, tag=f"a2a_in{w}", name=f"a2a_in{w}")
                a2a_out[w] = dpool.tile([NC, 128, 128], F8, tag=f"a2a_out{w}", name=f"a2a_out{w}")

            # warmup collective: absorbs the one-time CC ring setup while
            # phase A computes, so the first real AllToAll is fast
            warm_in = dpool.tile([NC, 1, 4], F32, tag="warm_in")
            warm_out = dpool.tile([NC, 1, 4], F32, tag="warm_out")
            warm_src = spool.tile([1, 4 * NC], F32, tag="warm_src", bufs=1)
            nc.vector.memset(warm_src[:], 0.0)
            nc.sync.dma_start(warm_in[:].rearrange("c p s -> p (c s)"), warm_src[:])
            nc.gpsimd.collective_compute(
                "AllToAll",
                BYPASS,
                replica_groups=[list(range(NC))],
                ins=[warm_in[:]],
                outs=[warm_out[:]],
            )

            # projection weights first on the SP queue (phase A needs them
            # immediately); cos/sin + tail weights ride the ACT hwdge queue
            wsb = {}
            bias_t = {}
            for name, dram_w, dram_b in (("q", wq, bqd), ("k", wk, bkd), ("v", wv, bvd)):
                wt = wpool.tile([128, 4, 2, 128], F8, tag=f"w{name}", name=f"w{name}")
                nc.sync.dma_start(wt[:].rearrange("p a b c -> p (a b c)"), dram_w[:])
                wsb[name] = wt
                bt = wpool.tile([128, 1], F32, tag=f"b{name}", name=f"b{name}")
                nc.sync.dma_start(bt[:, 0:1], dram_b[:])
                bias_t[name] = bt
            ident = wpool.tile([128, 128], F16, tag="ident")
            make_identity(nc, ident[:])
            cs2 = wpool.tile([128, S], F16, tag="cs2")
            sn2 = wpool.tile([128, S], F16, tag="sn2")
            nc.scalar.dma_start(cs2[:], cs2d[:])
            nc.scalar.dma_start(sn2[:], sn2d[:])
            won8 = wpool.tile([128, 2, 4, 2, 512], F8, tag="won8")
            nc.scalar.dma_start(won8[:].rearrange("p a b c d -> p (a b c d)"), wo8d[:])
            lnw = wpool.tile([128, D], F32, tag="lnw")
            lnb = wpool.tile([128, D], F32, tag="lnb")
            nc.scalar.dma_start(lnw[:], lnwd[:])
            nc.scalar.dma_start(lnb[:], lnbd[:])

            # ---- phase A: fp8 projections + RoPE + v transpose
            def phase_a(g0, g1):
                for g in range(g0, g1):
                    gs = 512 * g
                    cg = gs % S
                    xg = xpool.tile([128, 4, 2, 512], F8, tag="xg")
                    for cp in range(4):
                        for i in range(2):
                            nc.sync.dma_start(
                                xg[:, cp, i, :],
                                xT[256 * cp + 128 * i : 256 * cp + 128 * (i + 1), gs : gs + 512],
                            )
                    for name in ("q", "k", "v"):
                        pp = psC.tile([128, 512], F32, tag="cp", name="pp")
                        for cp in range(4):
                            nc.tensor.matmul(
                                pp[:],
                                wsb[name][:, cp, :, :],
                                xg[:, cp, :, :],
                                start=(cp == 0),
                                stop=(cp == 3),
                                perf_mode=DR,
                            )
                        plain = rpool.tile([128, 512], F16, tag="plain")
                        nc.scalar.activation(plain[:], pp[:], IDENT, bias=bias_t[name][:])
                        if name in ("q", "k"):
                            # in the permuted layout rotate_half is a clean
                            # +-64-partition shift done on the PSUM->SBUF
                            # evacuation; sn2 carries the sign
                            tc_ = rpool.tile([128, 512], F16, tag="tc")
                            nc.vector.tensor_tensor(
                                out=tc_[:], in0=plain[:], in1=cs2[:, cg : cg + 512], op=MULT
                            )
                            plrot = rpool.tile([128, 512], F16, tag="plrot")
                            for i in range(2):
                                nc.scalar.activation(
                                    plrot[64 * i : 64 * (i + 1), :],
                                    pp[64 * (1 - i) : 64 * (2 - i), :],
                                    IDENT,
                                    bias=bias_t[name][64 * (1 - i) : 64 * (2 - i), :],
                                )
                            ts_ = rpool.tile([128, 512], F16, tag="ts")
                            nc.vector.tensor_tensor(
                                out=ts_[:], in0=plrot[:], in1=sn2[:, cg : cg + 512], op=MULT
                            )
                            dst = qS8 if name == "q" else kS8
                            for i in range(2):
                                nc.gpsimd.tensor_tensor(
                                    out=dst[0:64, i, gs : gs + 512],
                                    in0=tc_[64 * i : 64 * (i + 1), :],
                                    in1=ts_[64 * i : 64 * (i + 1), :],
                                    op=ADD,
                                )
                        else:
                            for sub in range(4):
                                trp = psC.tile([128, 128], F16, tag="cp", name="trp")
                                nc.tensor.transpose(
                                    trp[:], plain[:, 128 * sub : 128 * (sub + 1)], ident[:]
                                )
                                tcg = 4 * g + sub
                                nc.vector.tensor_copy(
                                    vaug[:, tcg // 2, tcg % 2, :, 0:HD],
                                    trp[:].rearrange("p (h d) -> p h d", h=HPC),
                                )

            # ---- phase B: attention for window w, then normalize + AllToAll
            def phase_b(w):
                b = w // 2
                sw = 1024 * w
                cps = {}
                for h in range(HPC):
                    for half in range(2):
                        cps[h, half] = psC.tile(
                            [MV, 512], F32, tag="cp", name=f"cps_{h}_{half}"
                        )
                for pg in range(NPAIR):
                    ex8 = {}
                    for h in range(HPC):
                        ex8[h] = epool.tile([128, 2, 1024], F8, tag="ex", name=f"ex{h}")
                    for i in range(2):
                        tcl = 2 * pg + i
                        tg = S * b + 128 * tcl
                        for h in range(HPC):
                            sc = psS.tile([128, 1024], F32, tag="sc")
                            for half in range(2):
                                s0 = sw + 512 * half
                                nc.tensor.matmul(
                                    sc[:, 512 * half : 512 * (half + 1)],
                                    kS8[32 * h : 32 * (h + 1), :, tg : tg + 128],
                                    qS8[32 * h : 32 * (h + 1), :, s0 : s0 + 512],
                                    start=True,
                                    stop=True,
                                    perf_mode=DR,
                                )
                            nc.scalar.activation(
                                ex8[h][:, i, :], sc[:], EXP, scale=EXP_SCALE
                            )
                    pgg = NPAIR * b + pg
                    for h in range(HPC):
                        for half in range(2):
                            nc.tensor.matmul(
                                cps[h, half][:],
                                vaug[:, pgg, :, h, :],
                                ex8[h][:, :, 512 * half : 512 * (half + 1)],
                                start=(pg == 0),
                                stop=(pg == NPAIR - 1),
                                perf_mode=DR,
                            )
                # normalize ctx by the denominator row and ship via AllToAll
                ctile = ctxpool.tile([128, 1024], F8, tag="ctile")
                for h in range(HPC):
                    for half in range(2):
                        dsb = spool.tile([1, 512], F32, tag="dsb")
                        nc.vector.tensor_copy(dsb[:], cps[h, half][HD : HD + 1, :])
                        rden = spool.tile([1, 512], F32, tag="rden")
                        nc.vector.reciprocal_approx_fast(rden[:], dsb[:])
                        bc = bpool.tile([HD, 512], F32, tag="bc")
                        nc.gpsimd.partition_broadcast(bc[:], rden[:])
                        nc.vector.tensor_tensor(
                            out=ctile[HD * h : HD * (h + 1), 512 * half : 512 * (half + 1)],
                            in0=cps[h, half][0:HD, :],
                            in1=bc[:],
                            op=MULT,
                        )
                if _dump:
                    nc.sync.dma_start(cto[w], ctile[:])
                nc.sync.dma_start(
                    a2a_in[w][:].rearrange("c p s -> p c s"),
                    ctile[:].rearrange("p (c s) -> p c s", c=NC),
                )
                nc.gpsimd.collective_compute(
                    "AllToAll",
                    BYPASS,
                    replica_groups=[list(range(NC))],
                    ins=[a2a_in[w][:]],
                    outs=[a2a_out[w][:]],
                )
                rsb = lpool.tile([128, D], F32, tag="rsb", name=f"rsb{w}")
                nc.sync.dma_start(rsb[:], residd[128 * w : 128 * (w + 1), :])
                return rsb

            # ---- phase C: gather, out-proj, residual + LayerNorm, store
            def phase_c(w, rsb):
                # gather + output DMAs ride the ACT hwdge queue so they don't
                # serialize behind next-window a2a_in triggers on the SP queue
                ctxF = ctxpool.tile([128, 1024], F8, tag="ctxF")
                nc.scalar.dma_start(
                    ctxF[:].rearrange("p (c s) -> p c s", c=NC),
                    a2a_out[w][:].rearrange("c p s -> p c s"),
                )
                if _dump:
                    nc.sync.dma_start(cfo[w], ctxF[:])
                osb = lpool.tile([128, D], F32, tag="osb")
                for n in range(2):
                    op = psC.tile([128, 512], F32, tag="cp", name="op")
                    for cp in range(4):
                        nc.tensor.matmul(
                            op[:],
                            ctxF[:, 256 * cp : 256 * (cp + 1)].rearrange(
                                "p (i s) -> p i s", i=2
                            ),
                            won8[:, n, cp, :, :],
                            start=(cp == 0),
                            stop=(cp == 3),
                            perf_mode=DR,
                        )
                    nc.vector.tensor_tensor(
                        out=osb[:, 512 * n : 512 * (n + 1)],
                        in0=op[:],
                        in1=rsb[:, 512 * n : 512 * (n + 1)],
                        op=ADD,
                    )
                if _dump:
                    nc.sync.dma_start(oso[w], osb[:])
                # mean+var in three DVE ops via the BN statistics instructions
                stats = spool.tile([128, 12], F32, tag="stats")
                for n in range(2):
                    nc.vector.bn_stats(
                        stats[:, 6 * n : 6 * (n + 1)],
                        osb[:, 512 * n : 512 * (n + 1)],
                    )
                mv = spool.tile([128, 2], F32, tag="mv")
                nc.vector.bn_aggr(mv[:], stats[:])
                mean = mv[:, 0:1]
                var = mv[:, 1:2]
                # rstd = 1/sqrt(var): magic seed + 2 Newton steps — the whole
                # tail stays on the DVE (no cross-engine semaphore hops, and
                # the ACT engine never leaves the Exp table)
                ish = spool.tile([128, 1], I32, tag="ish")
                nc.vector.tensor_scalar(ish[:], var.bitcast(I32), 1, None, SHR)
                noti = spool.tile([128, 1], I32, tag="noti")
                nc.vector.tensor_scalar(noti[:], ish[:], -1, None, XOR)
                seed = spool.tile([128, 1], I32, tag="seed")
                nc.vector.tensor_scalar(seed[:], noti[:], RSQRT_MAGIC + 1, None, ADD)
                y = seed[:].bitcast(F32)
                for it in range(2):
                    a_ = spool.tile([128, 1], F32, tag=f"nra{it}", name=f"nra{it}")
                    nc.vector.tensor_tensor(out=a_[:], in0=y, in1=y, op=MULT)
                    b_ = spool.tile([128, 1], F32, tag=f"nrb{it}", name=f"nrb{it}")
                    nc.vector.tensor_tensor(out=b_[:], in0=a_[:], in1=var, op=MULT)
                    c_ = spool.tile([128, 1], F32, tag=f"nrc{it}", name=f"nrc{it}")
                    nc.vector.tensor_scalar(c_[:], b_[:], -0.5, 1.5, MULT, ADD)
                    y2 = spool.tile([128, 1], F32, tag=f"nry{it}", name=f"nry{it}")
                    nc.vector.tensor_tensor(out=y2[:], in0=y, in1=c_[:], op=MULT)
                    y = y2[:]
                mr = spool.tile([128, 1], F32, tag="mr")
                nc.vector.tensor_tensor(out=mr[:], in0=mean, in1=y, op=MULT)
                negmr = spool.tile([128, 1], F32, tag="negmr")
                nc.vector.tensor_scalar(negmr[:], mr[:], -1.0, None, MULT)
                onrm = lpool.tile([128, D], F32, tag="onrm")
                nc.vector.tensor_scalar(onrm[:], osb[:], y, negmr[:], MULT, ADD)
                ow = lpool.tile([128, D], F32, tag="ow")
                nc.vector.tensor_tensor(out=ow[:], in0=onrm[:], in1=lnw[:], op=MULT)
                ofin = lpool.tile([128, D], F32, tag="ofin")
                nc.vector.tensor_tensor(out=ofin[:], in0=ow[:], in1=lnb[:], op=ADD)
                nc.scalar.dma_start(outd[128 * w : 128 * (w + 1), :], ofin[:])

            # ---- schedule: interleave so exp/collectives overlap PE work and
            # each window's out-proj hides behind the next window's attention
            phase_a(0, 4)
            rsb0 = phase_b(0)
            rsb1 = phase_b(1)
            phase_a(4, 8)
            phase_c(0, rsb0)
            rsb2 = phase_b(2)
            phase_c(1, rsb1)
            rsb3 = phase_b(3)
            phase_c(2, rsb2)
            phase_c(3, rsb3)

    nc.finalize()
    return nc


# feature permutation: partition p of the projected q/k carries original
# head-feature fperm[p]; head h's d-half i lives at partitions [32h,32h+32)
# free-slot i, which is partition block 64i + 32h of the projection output
def _fperm():
    p = np.empty(128, np.int64)
    for h in range(HPC):
        for dd in range(32):
            p[32 * h + dd] = 64 * h + dd            # d_lo at blocks 0/1
            p[64 + 32 * h + dd] = 64 * h + 32 + dd  # d_hi at blocks 2/3
    return p


def _to_fp8(a):
    return np.ascontiguousarray(a.astype(ml_dtypes.float8_e4m3))


def kernel(hidden_states, cos, sin, Wq, bq, Wk, bk, Wv, bv, Wo, bo, ln_w, ln_b):
    global LAST_RESULTS
    hs = np.ascontiguousarray(np.asarray(hidden_states, np.float32).reshape(NSEQ, D))
    cos = np.asarray(cos, np.float32)
    sin = np.asarray(sin, np.float32)
    Wq = np.asarray(Wq, np.float32)
    bq = np.asarray(bq, np.float32)
    Wk = np.asarray(Wk, np.float32)
    bk = np.asarray(bk, np.float32)
    Wv = np.asarray(Wv, np.float32)
    bv = np.asarray(bv, np.float32)
    Wo = np.asarray(Wo, np.float32)
    bo = np.asarray(bo, np.float32)
    ln_w = np.asarray(ln_w, np.float32)
    ln_b = np.asarray(ln_b, np.float32)

    fperm = _fperm()
    xT8 = _to_fp8(hs.T)
    cosT = cos.T
    sinT = sin.T
    # row p carries cos/sin of d = fperm[p] % 64; sn2 folds the rotate-half
    # sign (rows 0:64 are the d_lo outputs, sourced from -q[d_hi])
    cs2 = np.ascontiguousarray(cosT[fperm % 64].astype(np.float16))
    sgn = np.where(np.arange(128) < 64, -1.0, 1.0)[:, None].astype(np.float32)
    sn2 = np.ascontiguousarray((sgn * sinT[fperm % 64]).astype(np.float16))
    lnw_t = np.ascontiguousarray(np.tile(ln_w[None, :], (128, 1)))
    lnb_t = np.ascontiguousarray(np.tile(ln_b[None, :], (128, 1)))
    # Wo scaled x4 into fp8's normal range, laid out for DoubleRow rhs
    wo = 4.0 * Wo.T                                   # [din, dout]
    wo8 = wo.reshape(4, 2, 128, 2, 512)               # [cp, i, p, n, dout]
    wo8 = wo8.transpose(2, 3, 0, 1, 4)                # [p, n, cp, i, dout]
    wo8 = _to_fp8(wo8.reshape(128, 2 * 4 * 2 * 512))

    def w8_layout(WT_cols):
        # WT_cols: [D, 128] = W.T slice for this core's features (scaled)
        # -> [128 part, (4 cpair, 2 half, 128 feat)] fp8
        a = WT_cols.reshape(4, 2, 128, 128)          # [cp, i, p, f]
        a = a.transpose(2, 0, 1, 3)                   # [p, cp, i, f]
        return _to_fp8(a.reshape(128, D))

    # x16: matches the 4*ctx fp8 ctile times the x4 fp8 Wo; LayerNorm is
    # scale-invariant so the output is unchanged
    resid_full = WSCALE * (hs + bo[None, :])

    in_maps = []
    for c in range(NC):
        fs = np.arange(F * c, F * (c + 1))
        fs_p = fs[0] + fperm                          # permuted q/k features
        rows = np.concatenate(
            [resid_full[1024 * w + 128 * c : 1024 * w + 128 * (c + 1)] for w in range(NW)],
            axis=0,
        )
        in_maps.append(
            {
                "xT": xT8,
                "wq": w8_layout(WSCALE * Wq.T[:, fs_p]),
                "wk": w8_layout(WSCALE * Wk.T[:, fs_p]),
                "wv": w8_layout(WSCALE * Wv.T[:, fs]),
                "bq": np.ascontiguousarray(WSCALE * bq[fs_p, None]),
                "bk": np.ascontiguousarray(WSCALE * bk[fs_p, None]),
                "bv": np.ascontiguousarray(WSCALE * bv[fs, None]),
                "wo8": wo8,
                "cs2": cs2,
                "sn2": sn2,
                "resid": np.ascontiguousarray(rows),
                "lnw": lnw_t,
                "lnb": lnb_t,
            }
        )

    nc = _build()
    LAST_RESULTS = run_bass_kernel_spmd(nc, in_maps, core_ids=list(range(NC)))
    out = np.empty((NSEQ, D), np.float32)
    for c in range(NC):
        res = LAST_RESULTS.results[c]["out"]
        for w in range(NW):
            out[1024 * w + 128 * c : 1024 * w + 128 * (c + 1)] = res[
                128 * w : 128 * (w + 1)
            ]
    return out.reshape(B, S, D)



# revision 8
# speedup vs baseline: 1.0012x; 1.0012x over previous
"""NomicBertAttention on 8 Trainium2 NeuronCores.

Sharding: 8-way head tensor-parallelism (2 heads/core, both batches).
Per 1024-column window of the flattened (b,s) axis, an fp8 AllToAll
re-shards ctx^T by sequence rows (each core owns one 128-row block per
window), and the row-parallel out-proj + residual + LayerNorm for that
window is software-pipelined behind the next window's attention.

Attention matmuls run in fp8e4m3 DoubleRow mode (fp32 PSUM):
projections contract K=256 per weight load, scores contract the 64-dim
head as 2x32 (head-dim halves interleaved in the free dim via a
host-side feature permutation), and ctx contracts 2 t-chunks (K=2x128)
with a ones-column in V producing the softmax denominator.  Scale
folding keeps fp8 in its normal range: W_{q,k,v} are scaled x16
host-side, the exp applies 1/(64*256), and Wo carries the rest.

Schedule: the PE runs in two tiling modes (32x128 for scores -- the two
heads land on different row groups and execute concurrently -- and
128x128 for everything else); emission groups same-mode matmuls and
lags each pair's ctx matmuls one pair behind its scores so the PE never
alternates modes per-instruction.  Projection groups for batch 1 and
the out-proj/LayerNorm phases interleave into the pair loop as
128-mode filler.  Dummy matmuls at t=0 (and during the final AllToAll
wait) keep the PE HAM clock-gate at 8/8.

exp is split across the ACT engine (native Exp to fp8) and the DVE
(Schraudolph: the fp8e4m3 bit pattern of exp(x) is round(x*8/ln2 +
55.656) as int8 -- one tensor_scalar from PSUM).  The softmax stays
exactly normalized either way because the ones-column denominator sums
the actual fp8 weights used.  The Pool engine (no PSUM port) carries
the SBUF-side elementwise: RoPE adds, denominator broadcast, LN tail
products.  LayerNorm stats use bn_stats/bn_aggr and a magic-constant
rsqrt on the DVE so the ACT engine never leaves the Exp table.
"""

import numpy as np
import ml_dtypes
import concourse.bacc as bacc
import concourse.mybir as mybir
import concourse.tile as tile
from concourse.bass_utils import run_bass_kernel_spmd
from concourse.masks import make_identity

F32 = mybir.dt.float32
F16 = mybir.dt.float16
F8 = mybir.dt.float8e4
I8 = mybir.dt.int8
I32 = mybir.dt.int32
DR = mybir.MatmulPerfMode.DoubleRow
MULT = mybir.AluOpType.mult
ADD = mybir.AluOpType.add
XOR = mybir.AluOpType.bitwise_xor
SHR = mybir.AluOpType.arith_shift_right
BYPASS = mybir.AluOpType.bypass
EXP = mybir.ActivationFunctionType.Exp
IDENT = mybir.ActivationFunctionType.Identity

B, S, D, H, HD = 2, 2048, 1024, 16, 64
NC = 8
HPC = H // NC          # 2 heads per core
F = HPC * HD           # 128 projected features per core
NSEQ = B * S           # 4096 flattened rows
ROWS = NSEQ // NC      # 512 output rows per core (4 blocks of 128)
NW = 4                 # 1024-column windows
TB = S // 128          # 16 t-chunks per batch
NPAIR = TB // 2        # 8 t-chunk pairs per batch
MV = 80                # ctx stationary free (64 v + 1 ones + 15 pad; mult of 16)
WSCALE = 16.0          # fp8 range scaling folded into Wq/Wk/Wv
EXP_SCALE = 1.0 / (64.0 * WSCALE * WSCALE)
LN2 = 0.6931471805599453
SCHRA_A = EXP_SCALE * 8.0 / LN2     # fp8e4m3 bits of exp(scale*x) ~= A*x + B
SCHRA_B = 56.0 - 0.344
EPS = 1e-12
RSQRT_MAGIC = 0x5F3759DF

# exp engine per (pair parity, 2*i+h): 'A' = ACT native exp, 'V' = DVE
# Schraudolph int8.  5:3 split keeps both engines near-equally loaded.
EXPPAT = (("A", "V", "A", "A"), ("A", "V", "A", "V"))

LAST_RESULTS = None


def _build():
    nc = bacc.Bacc("TRN2", target_bir_lowering=False, debug=False, num_devices=NC)

    xT = nc.dram_tensor("xT", [D, NSEQ], F8, kind="ExternalInput")
    wq = nc.dram_tensor("wq", [128, D], F8, kind="ExternalInput")
    wk = nc.dram_tensor("wk", [128, D], F8, kind="ExternalInput")
    wv = nc.dram_tensor("wv", [128, D], F8, kind="ExternalInput")
    bqd = nc.dram_tensor("bq", [F, 1], F32, kind="ExternalInput")
    bkd = nc.dram_tensor("bk", [F, 1], F32, kind="ExternalInput")
    bvd = nc.dram_tensor("bv", [F, 1], F32, kind="ExternalInput")
    wo8d = nc.dram_tensor("wo8", [128, 2 * 4 * 2 * 512], F8, kind="ExternalInput")
    cs2d = nc.dram_tensor("cs2", [128, S], F16, kind="ExternalInput")
    sn2d = nc.dram_tensor("sn2", [128, S], F16, kind="ExternalInput")
    residd = nc.dram_tensor("resid", [ROWS, D], F32, kind="ExternalInput")
    lnwd = nc.dram_tensor("lnw", [128, D], F32, kind="ExternalInput")
    lnbd = nc.dram_tensor("lnb", [128, D], F32, kind="ExternalInput")
    outd = nc.dram_tensor("out", [ROWS, D], F32, kind="ExternalOutput")
    import os as _os0
    _dump = bool(_os0.environ.get("KD_DUMP"))
    if _dump:
        q8o = nc.dram_tensor("q8o", [64, 2 * NSEQ], F8, kind="ExternalOutput")
        k8o = nc.dram_tensor("k8o", [64, 2 * NSEQ], F8, kind="ExternalOutput")
        v8o = nc.dram_tensor("v8o", [128, 16 * 2 * HPC * MV], F8, kind="ExternalOutput")
        rdeno = nc.dram_tensor("rdeno", [NW * HPC * 2, 512], F32, kind="ExternalOutput")
        cto = nc.dram_tensor("cto", [NW, 128, 1024], F8, kind="ExternalOutput")
        cfo = nc.dram_tensor("cfo", [NW, 128, 1024], F8, kind="ExternalOutput")
        oso = nc.dram_tensor("oso", [NW, 128, D], F32, kind="ExternalOutput")

    with tile.TileContext(nc) as tc:
        with (
            tc.tile_pool(name="qk", bufs=1) as qkpool,
            tc.tile_pool(name="wpool", bufs=1) as wpool,
            tc.tile_pool(name="xpool", bufs=2) as xpool,
            tc.tile_pool(name="rope", bufs=2) as rpool,
            tc.tile_pool(name="exps", bufs=4) as epool,
            tc.tile_pool(name="ctxp", bufs=2) as ctxpool,
            tc.tile_pool(name="bcastp", bufs=2) as bpool,
            tc.tile_pool(name="small", bufs=4) as spool,
            tc.tile_pool(name="tail", bufs=2) as lpool,
            tc.tile_pool(name="psS", bufs=2, space="PSUM") as psS,
            tc.tile_pool(name="psC", bufs=4, space="PSUM") as psC,
            tc.tile_pool(name="dram", bufs=1, space="DRAM") as dpool,
        ):
            # ---- HAM warm-up: dependency-free matmuls from t=0 so the PE
            # clock-gate reaches 8/8 before the first projection issues.
            junkw = wpool.tile([128, 128], F16, tag="junkw")
            nc.vector.memset(junkw[:], 0.0)
            junkm = wpool.tile([128, 512], F16, tag="junkm")
            nc.vector.memset(junkm[:], 0.0)

            def emit_warm(n, name):
                dps = psS.tile([128, 512], F32, tag="sc", name=name)
                for _ in range(n):
                    nc.tensor.matmul(dps[:], junkw[:], junkm[:], start=True, stop=True)

            emit_warm(36, "warm0")

            # ---- resident tensors
            # q/k RoPE'd fp8: [32h+dlo (64 parts), d-half, pos]
            qS8 = qkpool.tile([64, 2, NSEQ], F8, tag="qS8")
            kS8 = qkpool.tile([64, 2, NSEQ], F8, tag="kS8")
            # v natural fp8 + ones col (=4, folds a 1/4 into the denominator
            # so ctile lands at 4*ctx, matching the x4-scaled fp8 Wo)
            vaug = qkpool.tile([128, 2 * NPAIR, 2, HPC, MV], F8, tag="vaug")
            nc.vector.memset(vaug[:, :, :, :, HD : HD + 1], 4.0)
            nc.vector.memset(vaug[:, :, :, :, HD + 1 : MV], 0.0)

            a2a_in = {}
            a2a_out = {}
            for w in range(NW):
                a2a_in[w] = dpool.tile([NC, 128, 128], F8, tag=f"a2a_in{w}", name=f"a2a_in{w}")
                a2a_out[w] = dpool.tile([NC, 128, 128], F8, tag=f"a2a_out{w}", name=f"a2a_out{w}")

            # warmup collective: absorbs the one-time CC ring setup while
            # early compute runs, so the first real AllToAll is fast
            warm_in = dpool.tile([NC, 1, 4], F32, tag="warm_in")
            warm_out = dpool.tile([NC, 1, 4], F32, tag="warm_out")
            warm_src = spool.tile([1, 4 * NC], F32, tag="warm_src", bufs=1)
            nc.vector.memset(warm_src[:], 0.0)
            nc.sync.dma_start(warm_in[:].rearrange("c p s -> p (c s)"), warm_src[:])
            nc.gpsimd.collective_compute(
                "AllToAll",
                BYPASS,
                replica_groups=[list(range(NC))],
                ins=[warm_in[:]],
                outs=[warm_out[:]],
            )

            # projection weights first on the SP queue (phase A needs them
            # immediately); cos/sin + tail weights ride the ACT/DVE queues
            wsb = {}
            bias_t = {}
            for name, dram_w, dram_b in (("q", wq, bqd), ("k", wk, bkd), ("v", wv, bvd)):
                wt = wpool.tile([128, 4, 2, 128], F8, tag=f"w{name}", name=f"w{name}")
                nc.sync.dma_start(wt[:].rearrange("p a b c -> p (a b c)"), dram_w[:])
                wsb[name] = wt
                bt = wpool.tile([128, 1], F32, tag=f"b{name}", name=f"b{name}")
                nc.sync.dma_start(bt[:, 0:1], dram_b[:])
                bias_t[name] = bt
            ident = wpool.tile([128, 128], F16, tag="ident")
            make_identity(nc, ident[:])
            cs2 = wpool.tile([128, S], F16, tag="cs2")
            sn2 = wpool.tile([128, S], F16, tag="sn2")
            nc.scalar.dma_start(cs2[:], cs2d[:])
            nc.scalar.dma_start(sn2[:], sn2d[:])
            won8 = wpool.tile([128, 2, 4, 2, 512], F8, tag="won8")
            nc.scalar.dma_start(won8[:].rearrange("p a b c d -> p (a b c d)"), wo8d[:])
            lnw = wpool.tile([128, D], F32, tag="lnw")
            lnb = wpool.tile([128, D], F32, tag="lnb")
            nc.scalar.dma_start(lnw[:], lnwd[:])
            nc.scalar.dma_start(lnb[:], lnbd[:])
            # residuals for all four windows prefetched on the DVE queue
            rsb = {}
            for w in range(NW):
                rsb[w] = qkpool.tile([128, D], F32, tag=f"rsb{w}", name=f"rsb{w}")
                nc.gpsimd.dma_start(rsb[w][:], residd[128 * w : 128 * (w + 1), :])

            # ---- phase A: fp8 projections + RoPE + v transpose (one group =
            # 512 positions).  PE work is 128-mode; evacuations ride ACT with
            # the bias, cos/sin products ride the DVE in fp16 2x mode, the
            # recombine adds ride the Pool.
            def emit_A(g):
                gs = 512 * g
                cg = gs % S
                xg = xpool.tile([128, 4, 2, 512], F8, tag="xg")
                for cp in range(4):
                    for i in range(2):
                        nc.sync.dma_start(
                            xg[:, cp, i, :],
                            xT[256 * cp + 128 * i : 256 * cp + 128 * (i + 1), gs : gs + 512],
                        )
                pps = {}
                for nm in ("q", "k", "v"):
                    pp = psS.tile([128, 512], F32, tag="sc", name=f"pp{nm}")
                    for cp in range(4):
                        nc.tensor.matmul(
                            pp[:],
                            wsb[nm][:, cp, :, :],
                            xg[:, cp, :, :],
                            start=(cp == 0),
                            stop=(cp == 3),
                            perf_mode=DR,
                        )
                    pps[nm] = pp
                for nm in ("q", "k"):
                    pp = pps[nm]
                    plain = rpool.tile([128, 512], F16, tag="plain")
                    nc.scalar.activation(plain[:], pp[:], IDENT, bias=bias_t[nm][:])
                    # rotate-half is a +-64-partition shift on the PSUM->SBUF
                    # evacuation; sn2 carries the sign
                    plrot = rpool.tile([128, 512], F16, tag="plrot")
                    for i in range(2):
                        nc.scalar.activation(
                            plrot[64 * i : 64 * (i + 1), :],
                            pp[64 * (1 - i) : 64 * (2 - i), :],
                            IDENT,
                            bias=bias_t[nm][64 * (1 - i) : 64 * (2 - i), :],
                        )
                    tc_ = rpool.tile([128, 512], F16, tag="tc")
                    nc.vector.tensor_tensor(
                        out=tc_[:], in0=plain[:], in1=cs2[:, cg : cg + 512], op=MULT
                    )
                    ts_ = rpool.tile([128, 512], F16, tag="ts")
                    nc.vector.tensor_tensor(
                        out=ts_[:], in0=plrot[:], in1=sn2[:, cg : cg + 512], op=MULT
                    )
                    dst = qS8 if nm == "q" else kS8
                    for i in range(2):
                        nc.gpsimd.tensor_tensor(
                            out=dst[0:64, i, gs : gs + 512],
                            in0=tc_[64 * i : 64 * (i + 1), :],
                            in1=ts_[64 * i : 64 * (i + 1), :],
                            op=ADD,
                        )
                # v: evacuate, PE-transpose 128-blocks, copy into vaug
                plv = rpool.tile([128, 512], F16, tag="plv")
                nc.scalar.activation(plv[:], pps["v"][:], IDENT, bias=bias_t["v"][:])
                trp = psS.tile([128, 4, 128], F16, tag="sc", name="trp")
                for sub in range(4):
                    nc.tensor.transpose(
                        trp[:, sub, :], plv[:, 128 * sub : 128 * (sub + 1)], ident[:]
                    )
                for sub in range(4):
                    tcg = 4 * g + sub
                    nc.vector.tensor_copy(
                        vaug[:, tcg // 2, tcg % 2, :, 0:HD],
                        trp[:, sub, :].rearrange("p (h d) -> p h d", h=HPC),
                    )

            # ---- phase B machinery
            cur_cps = {}

            def alloc_cps():
                for h in range(HPC):
                    for half in range(2):
                        cur_cps[h, half] = psC.tile(
                            [MV, 512], F32, tag="cps", name=f"cps_{h}_{half}"
                        )

            def emit_scores(w, pg):
                """8 score MMs (32-mode, 2-band concurrent) + 4 exps."""
                b = w // 2
                sw = 1024 * w
                ex = epool.tile([128, HPC, 2, 1024], F8, tag="ex", name=f"ex_{w}_{pg}")
                for i in range(2):
                    tcl = 2 * pg + i
                    tg = S * b + 128 * tcl
                    sc = {}
                    for h in range(HPC):
                        sc[h] = psS.tile([128, 1024], F32, tag="sc", name=f"sc{h}")
                    # interleave heads so the two row-bands co-issue
                    for half in range(2):
                        for h in range(HPC):
                            s0 = sw + 512 * half
                            nc.tensor.matmul(
                                sc[h][:, 512 * half : 512 * (half + 1)],
                                kS8[32 * h : 32 * (h + 1), :, tg : tg + 128],
                                qS8[32 * h : 32 * (h + 1), :, s0 : s0 + 512],
                                start=True,
                                stop=True,
                                perf_mode=DR,
                            )
                    for h in range(HPC):
                        eng = EXPPAT[pg % 2][2 * i + h]
                        if eng == "A":
                            nc.scalar.activation(
                                ex[:, h, i, :], sc[h][:], EXP, scale=EXP_SCALE
                            )
                        else:
                            nc.vector.tensor_scalar(
                                ex[:, h, i, :].bitcast(I8),
                                sc[h][:],
                                SCHRA_A,
                                SCHRA_B,
                                MULT,
                                ADD,
                            )
                return ex

            def emit_ctx(w, pg, ex):
                b = w // 2
                pgg = NPAIR * b + pg
                for h in range(HPC):
                    for half in range(2):
                        nc.tensor.matmul(
                            cur_cps[h, half][:],
                            vaug[:, pgg, :, h, :],
                            ex[:, h, :, 512 * half : 512 * (half + 1)],
                            start=(pg == 0),
                            stop=(pg == NPAIR - 1),
                            perf_mode=DR,
                        )

            def emit_window(w, fillers):
                """Pair loop, ctx lagged one pair behind scores; `fillers`
                maps pair index -> closure emitting 128-mode work."""
                alloc_cps()
                prev = None
                for pg in range(NPAIR):
                    ex = emit_scores(w, pg)
                    if prev is not None:
                        emit_ctx(w, pg - 1, prev)
                    f = fillers.get(pg)
                    if f is not None:
                        f()
                    prev = ex
                emit_ctx(w, NPAIR - 1, prev)

            def emit_norm_a2a(w):
                # normalize ctx by the denominator row and ship via AllToAll
                ctile = ctxpool.tile([128, 1024], F8, tag="ctile")
                for h in range(HPC):
                    for half in range(2):
                        # den must be staged through SBUF: the custom-DVE
                        # reciprocal reads garbage from PSUM on hardware
                        dsb = spool.tile([1, 512], F32, tag="dsb")
                        nc.vector.tensor_copy(dsb[:], cur_cps[h, half][HD : HD + 1, :])
                        rden = spool.tile([1, 512], F32, tag="rden")
                        nc.vector.reciprocal_approx_fast(rden[:], dsb[:])
                        if _dump:
                            nc.sync.dma_start(
                                rdeno[(w * HPC + h) * 2 + half : (w * HPC + h) * 2 + half + 1, :],
                                rden[:],
                            )
                        bc = bpool.tile([HD, 512], F32, tag="bc")
                        nc.gpsimd.partition_broadcast(bc[:], rden[:])
                        nc.vector.tensor_tensor(
                            out=ctile[HD * h : HD * (h + 1), 512 * half : 512 * (half + 1)],
                            in0=cur_cps[h, half][0:HD, :],
                            in1=bc[:],
                            op=MULT,
                        )
                if _dump:
                    nc.sync.dma_start(cto[w], ctile[:])
                nc.sync.dma_start(
                    a2a_in[w][:].rearrange("c p s -> p c s"),
                    ctile[:].rearrange("p (c s) -> p c s", c=NC),
                )
                nc.gpsimd.collective_compute(
                    "AllToAll",
                    BYPASS,
                    replica_groups=[list(range(NC))],
                    ins=[a2a_in[w][:]],
                    outs=[a2a_out[w][:]],
                )

            # ---- phase C: gather, out-proj, residual + LayerNorm, store.
            # Split in two halves so it can interleave into the pair loop.
            def emit_C_mm(w):
                ctxF = ctxpool.tile([128, 1024], F8, tag="ctxF")
                nc.scalar.dma_start(
                    ctxF[:].rearrange("p (c s) -> p c s", c=NC),
                    a2a_out[w][:].rearrange("c p s -> p c s"),
                )
                if _dump:
                    nc.sync.dma_start(cfo[w], ctxF[:])
                osb = lpool.tile([128, D], F32, tag="osb")
                for n in range(2):
                    op = psS.tile([128, 512], F32, tag="sc", name="op")
                    for cp in range(4):
                        nc.tensor.matmul(
                            op[:],
                            ctxF[:, 256 * cp : 256 * (cp + 1)].rearrange(
                                "p (i s) -> p i s", i=2
                            ),
                            won8[:, n, cp, :, :],
                            start=(cp == 0),
                            stop=(cp == 3),
                            perf_mode=DR,
                        )
                    nc.vector.tensor_tensor(
                        out=osb[:, 512 * n : 512 * (n + 1)],
                        in0=op[:],
                        in1=rsb[w][:, 512 * n : 512 * (n + 1)],
                        op=ADD,
                    )
                return osb

            def emit_C_ln(w, osb):
                if _dump:
                    nc.sync.dma_start(oso[w], osb[:])
                # mean+var in three DVE ops via the BN statistics instructions
                stats = spool.tile([128, 12], F32, tag="stats")
                for n in range(2):
                    nc.vector.bn_stats(
                        stats[:, 6 * n : 6 * (n + 1)],
                        osb[:, 512 * n : 512 * (n + 1)],
                    )
                mv = spool.tile([128, 2], F32, tag="mv")
                nc.vector.bn_aggr(mv[:], stats[:])
                mean = mv[:, 0:1]
                var = mv[:, 1:2]
                # rstd = 1/sqrt(var): magic seed + 2 Newton steps on the DVE
                # (no cross-engine hops; ACT never leaves the Exp table)
                ish = spool.tile([128, 1], I32, tag="ish")
                nc.vector.tensor_scalar(ish[:], var.bitcast(I32), 1, None, SHR)
                noti = spool.tile([128, 1], I32, tag="noti")
                nc.vector.tensor_scalar(noti[:], ish[:], -1, None, XOR)
                seed = spool.tile([128, 1], I32, tag="seed")
                nc.vector.tensor_scalar(seed[:], noti[:], RSQRT_MAGIC + 1, None, ADD)
                y = seed[:].bitcast(F32)
                for it in range(2):
                    a_ = spool.tile([128, 1], F32, tag=f"nra{it}", name=f"nra{it}")
                    nc.vector.tensor_tensor(out=a_[:], in0=y, in1=y, op=MULT)
                    b_ = spool.tile([128, 1], F32, tag=f"nrb{it}", name=f"nrb{it}")
                    nc.vector.tensor_tensor(out=b_[:], in0=a_[:], in1=var, op=MULT)
                    c_ = spool.tile([128, 1], F32, tag=f"nrc{it}", name=f"nrc{it}")
                    nc.vector.tensor_scalar(c_[:], b_[:], -0.5, 1.5, MULT, ADD)
                    y2 = spool.tile([128, 1], F32, tag=f"nry{it}", name=f"nry{it}")
                    nc.vector.tensor_tensor(out=y2[:], in0=y, in1=c_[:], op=MULT)
                    y = y2[:]
                mr = spool.tile([128, 1], F32, tag="mr")
                nc.vector.tensor_tensor(out=mr[:], in0=mean, in1=y, op=MULT)
                negmr = spool.tile([128, 1], F32, tag="negmr")
                nc.vector.tensor_scalar(negmr[:], mr[:], -1.0, None, MULT)
                onrm = lpool.tile([128, D], F32, tag="onrm")
                nc.vector.tensor_scalar(onrm[:], osb[:], y, negmr[:], MULT, ADD)
                # lnw/lnb products on the Pool (SBUF-only engine)
                ow = lpool.tile([128, D], F32, tag="ow")
                nc.gpsimd.tensor_tensor(out=ow[:], in0=onrm[:], in1=lnw[:], op=MULT)
                ofin = lpool.tile([128, D], F32, tag="ofin")
                nc.gpsimd.tensor_tensor(out=ofin[:], in0=ow[:], in1=lnb[:], op=ADD)
                nc.scalar.dma_start(outd[128 * w : 128 * (w + 1), :], ofin[:])

            # ---- top-level schedule
            emit_A(0)
            emit_A(1)
            emit_window(0, {1: lambda: emit_A(2), 3: lambda: emit_A(3)})
            emit_norm_a2a(0)
            emit_window(
                1,
                {
                    0: lambda: emit_A(4),
                    2: lambda: emit_A(5),
                    4: lambda: emit_A(6),
                    6: lambda: emit_A(7),
                },
            )
            emit_norm_a2a(1)
            c_osb = {}

            def c_mm(w):
                c_osb[w] = emit_C_mm(w)

            emit_window(2, {1: lambda: c_mm(0), 4: lambda: emit_C_ln(0, c_osb[0])})
            emit_norm_a2a(2)
            emit_window(3, {1: lambda: c_mm(1), 4: lambda: emit_C_ln(1, c_osb[1])})
            emit_norm_a2a(3)
            c_mm(2)
            emit_C_ln(2, c_osb[2])
            emit_warm(14, "warmtail")
            c_mm(3)
            emit_C_ln(3, c_osb[3])
            if _dump:
                nc.sync.dma_start(q8o[:], qS8[:].rearrange("p a b -> p (a b)"))
                nc.sync.dma_start(k8o[:], kS8[:].rearrange("p a b -> p (a b)"))
                nc.sync.dma_start(v8o[:], vaug[:].rearrange("p a b c d -> p (a b c d)"))

    nc.finalize()
    return nc


# feature permutation: partition p of the projected q/k carries original
# head-feature fperm[p]; head h's d-half i lives at partitions [32h,32h+32)
# free-slot i, which is partition block 64i + 32h of the projection output
def _fperm():
    p = np.empty(128, np.int64)
    for h in range(HPC):
        for dd in range(32):
            p[32 * h + dd] = 64 * h + dd            # d_lo at blocks 0/1
            p[64 + 32 * h + dd] = 64 * h + 32 + dd  # d_hi at blocks 2/3
    return p


def _to_fp8(a):
    return np.ascontiguousarray(a.astype(ml_dtypes.float8_e4m3))


def _prep_in_maps(hidden_states, cos, sin, Wq, bq, Wk, bk, Wv, bv, Wo, bo, ln_w, ln_b):
    hs = np.ascontiguousarray(np.asarray(hidden_states, np.float32).reshape(NSEQ, D))
    cos = np.asarray(cos, np.float32)
    sin = np.asarray(sin, np.float32)
    Wq = np.asarray(Wq, np.float32)
    bq = np.asarray(bq, np.float32)
    Wk = np.asarray(Wk, np.float32)
    bk = np.asarray(bk, np.float32)
    Wv = np.asarray(Wv, np.float32)
    bv = np.asarray(bv, np.float32)
    Wo = np.asarray(Wo, np.float32)
    bo = np.asarray(bo, np.float32)
    ln_w = np.asarray(ln_w, np.float32)
    ln_b = np.asarray(ln_b, np.float32)

    fperm = _fperm()
    xT8 = _to_fp8(hs.T)
    cosT = cos.T
    sinT = sin.T
    # row p carries cos/sin of d = fperm[p] % 64; sn2 folds the rotate-half
    # sign (rows 0:64 are the d_lo outputs, sourced from -q[d_hi])
    cs2 = np.ascontiguousarray(cosT[fperm % 64].astype(np.float16))
    sgn = np.where(np.arange(128) < 64, -1.0, 1.0)[:, None].astype(np.float32)
    sn2 = np.ascontiguousarray((sgn * sinT[fperm % 64]).astype(np.float16))
    lnw_t = np.ascontiguousarray(np.tile(ln_w[None, :], (128, 1)))
    lnb_t = np.ascontiguousarray(np.tile(ln_b[None, :], (128, 1)))
    # Wo scaled x4 into fp8's normal range, laid out for DoubleRow rhs
    wo = 4.0 * Wo.T                                   # [din, dout]
    wo8 = wo.reshape(4, 2, 128, 2, 512)               # [cp, i, p, n, dout]
    wo8 = wo8.transpose(2, 3, 0, 1, 4)                # [p, n, cp, i, dout]
    wo8 = _to_fp8(wo8.reshape(128, 2 * 4 * 2 * 512))

    def w8_layout(WT_cols):
        # WT_cols: [D, 128] = W.T slice for this core's features (scaled)
        # -> [128 part, (4 cpair, 2 half, 128 feat)] fp8
        a = WT_cols.reshape(4, 2, 128, 128)          # [cp, i, p, f]
        a = a.transpose(2, 0, 1, 3)                   # [p, cp, i, f]
        return _to_fp8(a.reshape(128, D))

    # x16: matches the 4*ctx fp8 ctile times the x4 fp8 Wo; LayerNorm is
    # scale-invariant so the output is unchanged
    resid_full = WSCALE * (hs + bo[None, :])

    in_maps = []
    for c in range(NC):
        fs = np.arange(F * c, F * (c + 1))
        fs_p = fs[0] + fperm                          # permuted q/k features
        rows = np.concatenate(
            [resid_full[1024 * w + 128 * c : 1024 * w + 128 * (c + 1)] for w in range(NW)],
            axis=0,
        )
        in_maps.append(
            {
                "xT": xT8,
                "wq": w8_layout(WSCALE * Wq.T[:, fs_p]),
                "wk": w8_layout(WSCALE * Wk.T[:, fs_p]),
                "wv": w8_layout(WSCALE * Wv.T[:, fs]),
                "bq": np.ascontiguousarray(WSCALE * bq[fs_p, None]),
                "bk": np.ascontiguousarray(WSCALE * bk[fs_p, None]),
                "bv": np.ascontiguousarray(WSCALE * bv[fs, None]),
                "wo8": wo8,
                "cs2": cs2,
                "sn2": sn2,
                "resid": np.ascontiguousarray(rows),
                "lnw": lnw_t,
                "lnb": lnb_t,
            }
        )
    return in_maps


def _unshard(results):
    out = np.empty((NSEQ, D), np.float32)
    for c in range(NC):
        res = results[c]["out"]
        for w in range(NW):
            out[1024 * w + 128 * c : 1024 * w + 128 * (c + 1)] = res[
                128 * w : 128 * (w + 1)
            ]
    return out.reshape(B, S, D)


def kernel(hidden_states, cos, sin, Wq, bq, Wk, bk, Wv, bv, Wo, bo, ln_w, ln_b):
    global LAST_RESULTS
    in_maps = _prep_in_maps(
        hidden_states, cos, sin, Wq, bq, Wk, bk, Wv, bv, Wo, bo, ln_w, ln_b
    )
    nc = _build()
    LAST_RESULTS = run_bass_kernel_spmd(nc, in_maps, core_ids=list(range(NC)))
    out = np.empty((NSEQ, D), np.float32)
    for c in range(NC):
        res = LAST_RESULTS.results[c]["out"]
        for w in range(NW):
            out[1024 * w + 128 * c : 1024 * w + 128 * (c + 1)] = res[
                128 * w : 128 * (w + 1)
            ]
    return out.reshape(B, S, D)


# revision 14
# speedup vs baseline: 1.0173x; 1.0161x over previous
"""NomicBertAttention on 8 Trainium2 NeuronCores.

Sharding: 8-way head tensor-parallelism (2 heads/core, both batches).
Per 1024-column window of the flattened (b,s) axis, an fp8 AllToAll
re-shards ctx^T by sequence rows (each core owns one 128-row block per
window), and the row-parallel out-proj + residual + LayerNorm for that
window is software-pipelined behind the next window's attention.

Attention matmuls run in fp8e4m3 DoubleRow mode (fp32 PSUM):
projections contract K=256 per weight load, scores contract the 64-dim
head as 2x32 (head-dim halves interleaved in the free dim via a
host-side feature permutation), and ctx contracts 2 t-chunks (K=2x128)
with a ones-column in V producing the softmax denominator.  Scale
folding keeps fp8 in its normal range: W_{q,k,v} are scaled x16
host-side, the exp applies 1/(64*256), and Wo carries the rest.

Schedule: the PE runs in two tiling modes (32x128 for scores -- the two
heads land on different row groups and execute concurrently -- and
128x128 for everything else); emission groups same-mode matmuls and
lags each pair's ctx matmuls one pair behind its scores so the PE never
alternates modes per-instruction.  Projection groups for batch 1 and
the out-proj/LayerNorm phases interleave into the pair loop as
128-mode filler.  Dummy matmuls at t=0 (and during the final AllToAll
wait) keep the PE HAM clock-gate at 8/8.

exp is split across the ACT engine (native Exp to fp8) and the DVE
(Schraudolph: the fp8e4m3 bit pattern of exp(x) is round(x*8/ln2 +
55.656) as int8 -- one tensor_scalar from PSUM).  The softmax stays
exactly normalized either way because the ones-column denominator sums
the actual fp8 weights used.  The Pool engine (no PSUM port) carries
the SBUF-side elementwise: RoPE adds, denominator broadcast, LN tail
products.  LayerNorm stats use bn_stats/bn_aggr and a magic-constant
rsqrt on the DVE so the ACT engine never leaves the Exp table.
"""

import numpy as np
import ml_dtypes
import concourse.bacc as bacc
import concourse.mybir as mybir
import concourse.tile as tile
from concourse.bass_utils import run_bass_kernel_spmd
from concourse.masks import make_identity

F32 = mybir.dt.float32
F16 = mybir.dt.float16
F8 = mybir.dt.float8e4
I8 = mybir.dt.int8
I32 = mybir.dt.int32
DR = mybir.MatmulPerfMode.DoubleRow
MULT = mybir.AluOpType.mult
ADD = mybir.AluOpType.add
XOR = mybir.AluOpType.bitwise_xor
SHR = mybir.AluOpType.arith_shift_right
BYPASS = mybir.AluOpType.bypass
EXP = mybir.ActivationFunctionType.Exp
IDENT = mybir.ActivationFunctionType.Identity

B, S, D, H, HD = 2, 2048, 1024, 16, 64
NC = 8
HPC = H // NC          # 2 heads per core
F = HPC * HD           # 128 projected features per core
NSEQ = B * S           # 4096 flattened rows
ROWS = NSEQ // NC      # 512 output rows per core (4 blocks of 128)
NW = 4                 # 1024-column windows
TB = S // 128          # 16 t-chunks per batch
NPAIR = TB // 2        # 8 t-chunk pairs per batch
MV = 80                # ctx stationary free (64 v + 1 ones + 15 pad; mult of 16)
WSCALE = 16.0          # fp8 range scaling folded into Wq/Wk/Wv
EXP_SCALE = 1.0 / (64.0 * WSCALE * WSCALE)
LN2 = 0.6931471805599453
SCHRA_A = EXP_SCALE * 8.0 / LN2     # fp8e4m3 bits of exp(scale*x) ~= A*x + B
SCHRA_B = 56.0 - 0.344
EPS = 1e-12
RSQRT_MAGIC = 0x5F3759DF

# exp engine per (pair parity, 2*i+h): 'A' = ACT native exp, 'V' = DVE
# Schraudolph int8.  5:3 split keeps both engines near-equally loaded.
EXPPAT = (("A", "V", "A", "A"), ("A", "V", "A", "V"))

LAST_RESULTS = None


def _build():
    nc = bacc.Bacc("TRN2", target_bir_lowering=False, debug=False, num_devices=NC)

    xT = nc.dram_tensor("xT", [D, NSEQ], F8, kind="ExternalInput")
    wq = nc.dram_tensor("wq", [128, D], F8, kind="ExternalInput")
    wk = nc.dram_tensor("wk", [128, D], F8, kind="ExternalInput")
    wv = nc.dram_tensor("wv", [128, D], F8, kind="ExternalInput")
    bqd = nc.dram_tensor("bq", [F, 1], F32, kind="ExternalInput")
    bkd = nc.dram_tensor("bk", [F, 1], F32, kind="ExternalInput")
    bvd = nc.dram_tensor("bv", [F, 1], F32, kind="ExternalInput")
    wo8d = nc.dram_tensor("wo8", [128, 2 * 4 * 2 * 512], F8, kind="ExternalInput")
    cs2d = nc.dram_tensor("cs2", [128, S], F16, kind="ExternalInput")
    sn2d = nc.dram_tensor("sn2", [128, S], F16, kind="ExternalInput")
    residd = nc.dram_tensor("resid", [ROWS, D], F32, kind="ExternalInput")
    lnwd = nc.dram_tensor("lnw", [128, D], F32, kind="ExternalInput")
    lnbd = nc.dram_tensor("lnb", [128, D], F32, kind="ExternalInput")
    outd = nc.dram_tensor("out", [ROWS, D], F32, kind="ExternalOutput")
    import os as _os0
    _dump = bool(_os0.environ.get("KD_DUMP"))
    if _dump:
        q8o = nc.dram_tensor("q8o", [64, 2 * NSEQ], F8, kind="ExternalOutput")
        k8o = nc.dram_tensor("k8o", [64, 2 * NSEQ], F8, kind="ExternalOutput")
        v8o = nc.dram_tensor("v8o", [128, 16 * 2 * HPC * MV], F8, kind="ExternalOutput")
        rdeno = nc.dram_tensor("rdeno", [NW * HPC * 2, 512], F32, kind="ExternalOutput")
        cto = nc.dram_tensor("cto", [NW, 128, 1024], F8, kind="ExternalOutput")
        cfo = nc.dram_tensor("cfo", [NW, 128, 1024], F8, kind="ExternalOutput")
        oso = nc.dram_tensor("oso", [NW, 128, D], F32, kind="ExternalOutput")

    with tile.TileContext(nc) as tc:
        with (
            tc.tile_pool(name="qk", bufs=1) as qkpool,
            tc.tile_pool(name="wpool", bufs=1) as wpool,
            tc.tile_pool(name="xpool", bufs=2) as xpool,
            tc.tile_pool(name="rope", bufs=2) as rpool,
            tc.tile_pool(name="exps", bufs=4) as epool,
            tc.tile_pool(name="ctxp", bufs=2) as ctxpool,
            tc.tile_pool(name="bcastp", bufs=2) as bpool,
            tc.tile_pool(name="small", bufs=4) as spool,
            tc.tile_pool(name="tail", bufs=2) as lpool,
            tc.tile_pool(name="psS", bufs=2, space="PSUM") as psS,
            tc.tile_pool(name="psC", bufs=4, space="PSUM") as psC,
            tc.tile_pool(name="dram", bufs=1, space="DRAM") as dpool,
        ):
            # ---- HAM warm-up: dependency-free matmuls from t=0 so the PE
            # clock-gate reaches 8/8 before the first projection issues.
            junkw = wpool.tile([128, 128], F16, tag="junkw")
            nc.vector.memset(junkw[:], 0.0)
            junkm = wpool.tile([128, 512], F16, tag="junkm")
            nc.vector.memset(junkm[:], 0.0)

            def emit_warm(n, name):
                dps = psS.tile([128, 512], F32, tag="sc", name=name)
                for _ in range(n):
                    nc.tensor.matmul(dps[:], junkw[:], junkm[:], start=True, stop=True)

            emit_warm(36, "warm0")

            # ---- resident tensors
            # q/k RoPE'd fp8: [32h+dlo, d-half, pos].  q is padded to 128
            # partitions (rows 64-127 zero) and k is stored per-head with the
            # other head's rows zeroed (kz[h]), so score matmuls contract the
            # full 128 partitions and stay in the PE's 128x128 tiling mode --
            # no mode switches (= forced drains) against the other matmuls.
            qS8 = qkpool.tile([128, 2, NSEQ], F8, tag="qS8")
            for p0 in range(64, 128, 32):
                nc.vector.memset(qS8[p0 : p0 + 32, :, :], 0.0)
            kz = {}
            for h in range(HPC):
                kz[h] = qkpool.tile([128, 2, NSEQ], F8, tag=f"kz{h}", name=f"kz{h}")
                for p0 in range(0, 128, 32):
                    if p0 != 32 * h:
                        nc.vector.memset(kz[h][p0 : p0 + 32, :, :], 0.0)
            # v natural fp8 + ones col (=4, folds a 1/4 into the denominator
            # so ctile lands at 4*ctx, matching the x4-scaled fp8 Wo)
            vaug = qkpool.tile([128, 2 * NPAIR, 2, HPC, MV], F8, tag="vaug")
            nc.vector.memset(vaug[:, :, :, :, HD : HD + 1], 4.0)
            nc.vector.memset(vaug[:, :, :, :, HD + 1 : MV], 0.0)

            a2a_in = {}
            a2a_out = {}
            for w in range(NW):
                a2a_in[w] = dpool.tile([NC, 128, 128], F8, tag=f"a2a_in{w}", name=f"a2a_in{w}")
                a2a_out[w] = dpool.tile([NC, 128, 128], F8, tag=f"a2a_out{w}", name=f"a2a_out{w}")

            # warmup collective: absorbs the one-time CC ring setup while
            # early compute runs, so the first real AllToAll is fast
            warm_in = dpool.tile([NC, 1, 4], F32, tag="warm_in")
            warm_out = dpool.tile([NC, 1, 4], F32, tag="warm_out")
            warm_src = spool.tile([1, 4 * NC], F32, tag="warm_src", bufs=1)
            nc.vector.memset(warm_src[:], 0.0)
            nc.sync.dma_start(warm_in[:].rearrange("c p s -> p (c s)"), warm_src[:])
            nc.gpsimd.collective_compute(
                "AllToAll",
                BYPASS,
                replica_groups=[list(range(NC))],
                ins=[warm_in[:]],
                outs=[warm_out[:]],
            )

            # projection weights first on the SP queue (phase A needs them
            # immediately); cos/sin + tail weights ride the ACT/DVE queues
            wsb = {}
            bias_t = {}
            for name, dram_w, dram_b in (("q", wq, bqd), ("k", wk, bkd), ("v", wv, bvd)):
                wt = wpool.tile([128, 4, 2, 128], F8, tag=f"w{name}", name=f"w{name}")
                nc.sync.dma_start(wt[:].rearrange("p a b c -> p (a b c)"), dram_w[:])
                wsb[name] = wt
                bt = wpool.tile([128, 1], F32, tag=f"b{name}", name=f"b{name}")
                nc.sync.dma_start(bt[:, 0:1], dram_b[:])
                bias_t[name] = bt
            ident = wpool.tile([128, 128], F16, tag="ident")
            make_identity(nc, ident[:])
            cs2 = wpool.tile([128, S], F16, tag="cs2")
            sn2 = wpool.tile([128, S], F16, tag="sn2")
            nc.scalar.dma_start(cs2[:], cs2d[:])
            nc.scalar.dma_start(sn2[:], sn2d[:])
            won8 = wpool.tile([128, 2, 4, 2, 512], F8, tag="won8")
            nc.scalar.dma_start(won8[:].rearrange("p a b c d -> p (a b c d)"), wo8d[:])
            lnw = wpool.tile([128, D], F32, tag="lnw")
            lnb = wpool.tile([128, D], F32, tag="lnb")
            nc.scalar.dma_start(lnw[:], lnwd[:])
            nc.scalar.dma_start(lnb[:], lnbd[:])
            # residuals for all four windows prefetched on the DVE queue
            rsb = {}
            for w in range(NW):
                rsb[w] = qkpool.tile([128, D], F32, tag=f"rsb{w}", name=f"rsb{w}")
                nc.gpsimd.dma_start(rsb[w][:], residd[128 * w : 128 * (w + 1), :])

            # ---- phase A: fp8 projections + RoPE + v transpose (one group =
            # 512 positions).  PE work is 128-mode; evacuations ride ACT with
            # the bias, cos/sin products ride the DVE in fp16 2x mode, the
            # recombine adds ride the Pool.
            def emit_A(g):
                gs = 512 * g
                cg = gs % S
                xg = xpool.tile([128, 4, 2, 512], F8, tag="xg")
                for cp in range(4):
                    for i in range(2):
                        nc.sync.dma_start(
                            xg[:, cp, i, :],
                            xT[256 * cp + 128 * i : 256 * cp + 128 * (i + 1), gs : gs + 512],
                        )
                pps = {}
                for nm in ("q", "k", "v"):
                    pp = psS.tile([128, 512], F32, tag="sc", name=f"pp{nm}")
                    for cp in range(4):
                        nc.tensor.matmul(
                            pp[:],
                            wsb[nm][:, cp, :, :],
                            xg[:, cp, :, :],
                            start=(cp == 0),
                            stop=(cp == 3),
                            perf_mode=DR,
                        )
                    pps[nm] = pp
                for nm in ("q", "k"):
                    pp = pps[nm]
                    plain = rpool.tile([128, 512], F16, tag="plain")
                    nc.scalar.activation(plain[:], pp[:], IDENT, bias=bias_t[nm][:])
                    # rotate-half is a +-64-partition shift on the PSUM->SBUF
                    # evacuation; sn2 carries the sign
                    plrot = rpool.tile([128, 512], F16, tag="plrot")
                    for i in range(2):
                        nc.scalar.activation(
                            plrot[64 * i : 64 * (i + 1), :],
                            pp[64 * (1 - i) : 64 * (2 - i), :],
                            IDENT,
                            bias=bias_t[nm][64 * (1 - i) : 64 * (2 - i), :],
                        )
                    tc_ = rpool.tile([128, 512], F16, tag="tc")
                    nc.vector.tensor_tensor(
                        out=tc_[:], in0=plain[:], in1=cs2[:, cg : cg + 512], op=MULT
                    )
                    ts_ = rpool.tile([128, 512], F16, tag="ts")
                    nc.vector.tensor_tensor(
                        out=ts_[:], in0=plrot[:], in1=sn2[:, cg : cg + 512], op=MULT
                    )
                    if nm == "q":
                        for i in range(2):
                            nc.gpsimd.tensor_tensor(
                                out=qS8[0:64, i, gs : gs + 512],
                                in0=tc_[64 * i : 64 * (i + 1), :],
                                in1=ts_[64 * i : 64 * (i + 1), :],
                                op=ADD,
                            )
                    else:
                        for i in range(2):
                            for h in range(HPC):
                                p0 = 64 * i + 32 * h
                                nc.gpsimd.tensor_tensor(
                                    out=kz[h][32 * h : 32 * (h + 1), i, gs : gs + 512],
                                    in0=tc_[p0 : p0 + 32, :],
                                    in1=ts_[p0 : p0 + 32, :],
                                    op=ADD,
                                )
                # v: evacuate, PE-transpose 128-blocks, copy into vaug
                plv = rpool.tile([128, 512], F16, tag="plv")
                nc.scalar.activation(plv[:], pps["v"][:], IDENT, bias=bias_t["v"][:])
                trp = psS.tile([128, 4, 128], F16, tag="sc", name="trp")
                for sub in range(4):
                    nc.tensor.transpose(
                        trp[:, sub, :], plv[:, 128 * sub : 128 * (sub + 1)], ident[:]
                    )
                for sub in range(4):
                    tcg = 4 * g + sub
                    nc.vector.tensor_copy(
                        vaug[:, tcg // 2, tcg % 2, :, 0:HD],
                        trp[:, sub, :].rearrange("p (h d) -> p h d", h=HPC),
                    )

            # ---- phase B machinery
            cur_cps = {}

            def alloc_cps():
                for h in range(HPC):
                    for half in range(2):
                        cur_cps[h, half] = psC.tile(
                            [MV, 512], F32, tag="cps", name=f"cps_{h}_{half}"
                        )

            def emit_scores(w, pg):
                """8 score MMs (32-mode, 2-band concurrent) + 4 exps."""
                b = w // 2
                sw = 1024 * w
                ex = epool.tile([128, HPC, 2, 1024], F8, tag="ex", name=f"ex_{w}_{pg}")
                for i in range(2):
                    tcl = 2 * pg + i
                    tg = S * b + 128 * tcl
                    sc = {}
                    for h in range(HPC):
                        sc[h] = psS.tile([128, 1024], F32, tag="sc", name=f"sc{h}")
                    for half in range(2):
                        for h in range(HPC):
                            s0 = sw + 512 * half
                            nc.tensor.matmul(
                                sc[h][:, 512 * half : 512 * (half + 1)],
                                kz[h][:, :, tg : tg + 128],
                                qS8[:, :, s0 : s0 + 512],
                                start=True,
                                stop=True,
                                perf_mode=DR,
                            )
                    for h in range(HPC):
                        eng = EXPPAT[pg % 2][2 * i + h]
                        if eng == "A":
                            nc.scalar.activation(
                                ex[:, h, i, :], sc[h][:], EXP, scale=EXP_SCALE
                            )
                        else:
                            nc.vector.tensor_scalar(
                                ex[:, h, i, :].bitcast(I8),
                                sc[h][:],
                                SCHRA_A,
                                SCHRA_B,
                                MULT,
                                ADD,
                            )
                return ex

            def emit_ctx(w, pg, ex):
                b = w // 2
                pgg = NPAIR * b + pg
                for h in range(HPC):
                    for half in range(2):
                        nc.tensor.matmul(
                            cur_cps[h, half][:],
                            vaug[:, pgg, :, h, :],
                            ex[:, h, :, 512 * half : 512 * (half + 1)],
                            start=(pg == 0),
                            stop=(pg == NPAIR - 1),
                            perf_mode=DR,
                        )

            def emit_window(w, fillers):
                """Pair loop, ctx lagged one pair behind scores; `fillers`
                maps pair index -> closure emitting 128-mode work."""
                alloc_cps()
                prev = None
                for pg in range(NPAIR):
                    ex = emit_scores(w, pg)
                    if prev is not None:
                        emit_ctx(w, pg - 1, prev)
                    f = fillers.get(pg)
                    if f is not None:
                        f()
                    prev = ex
                emit_ctx(w, NPAIR - 1, prev)

            def emit_norm_a2a(w):
                # normalize ctx by the denominator row and ship via AllToAll
                ctile = ctxpool.tile([128, 1024], F8, tag="ctile")
                for h in range(HPC):
                    for half in range(2):
                        # den must be staged through SBUF: the custom-DVE
                        # reciprocal reads garbage from PSUM on hardware
                        dsb = spool.tile([1, 512], F32, tag="dsb")
                        nc.vector.tensor_copy(dsb[:], cur_cps[h, half][HD : HD + 1, :])
                        rden = spool.tile([1, 512], F32, tag="rden")
                        nc.vector.reciprocal_approx_fast(rden[:], dsb[:])
                        if _dump:
                            nc.sync.dma_start(
                                rdeno[(w * HPC + h) * 2 + half : (w * HPC + h) * 2 + half + 1, :],
                                rden[:],
                            )
                        bc = bpool.tile([HD, 512], F32, tag="bc")
                        nc.gpsimd.partition_broadcast(bc[:], rden[:])
                        nc.vector.tensor_tensor(
                            out=ctile[HD * h : HD * (h + 1), 512 * half : 512 * (half + 1)],
                            in0=cur_cps[h, half][0:HD, :],
                            in1=bc[:],
                            op=MULT,
                        )
                if _dump:
                    nc.sync.dma_start(cto[w], ctile[:])
                nc.sync.dma_start(
                    a2a_in[w][:].rearrange("c p s -> p c s"),
                    ctile[:].rearrange("p (c s) -> p c s", c=NC),
                )
                nc.gpsimd.collective_compute(
                    "AllToAll",
                    BYPASS,
                    replica_groups=[list(range(NC))],
                    ins=[a2a_in[w][:]],
                    outs=[a2a_out[w][:]],
                )

            # ---- phase C: gather, out-proj, residual + LayerNorm, store.
            # Split in two halves so it can interleave into the pair loop.
            def emit_C_mm(w):
                ctxF = ctxpool.tile([128, 1024], F8, tag="ctxF")
                nc.scalar.dma_start(
                    ctxF[:].rearrange("p (c s) -> p c s", c=NC),
                    a2a_out[w][:].rearrange("c p s -> p c s"),
                )
                if _dump:
                    nc.sync.dma_start(cfo[w], ctxF[:])
                osb = lpool.tile([128, D], F32, tag="osb")
                for n in range(2):
                    op = psS.tile([128, 512], F32, tag="sc", name="op")
                    for cp in range(4):
                        nc.tensor.matmul(
                            op[:],
                            ctxF[:, 256 * cp : 256 * (cp + 1)].rearrange(
                                "p (i s) -> p i s", i=2
                            ),
                            won8[:, n, cp, :, :],
                            start=(cp == 0),
                            stop=(cp == 3),
                            perf_mode=DR,
                        )
                    nc.vector.tensor_tensor(
                        out=osb[:, 512 * n : 512 * (n + 1)],
                        in0=op[:],
                        in1=rsb[w][:, 512 * n : 512 * (n + 1)],
                        op=ADD,
                    )
                return osb

            def emit_C_ln(w, osb):
                if _dump:
                    nc.sync.dma_start(oso[w], osb[:])
                # mean+var in three DVE ops via the BN statistics instructions
                stats = spool.tile([128, 12], F32, tag="stats")
                for n in range(2):
                    nc.vector.bn_stats(
                        stats[:, 6 * n : 6 * (n + 1)],
                        osb[:, 512 * n : 512 * (n + 1)],
                    )
                mv = spool.tile([128, 2], F32, tag="mv")
                nc.vector.bn_aggr(mv[:], stats[:])
                mean = mv[:, 0:1]
                var = mv[:, 1:2]
                # rstd = 1/sqrt(var): magic seed + 2 Newton steps on the DVE
                # (no cross-engine hops; ACT never leaves the Exp table)
                ish = spool.tile([128, 1], I32, tag="ish")
                nc.vector.tensor_scalar(ish[:], var.bitcast(I32), 1, None, SHR)
                noti = spool.tile([128, 1], I32, tag="noti")
                nc.vector.tensor_scalar(noti[:], ish[:], -1, None, XOR)
                seed = spool.tile([128, 1], I32, tag="seed")
                nc.vector.tensor_scalar(seed[:], noti[:], RSQRT_MAGIC + 1, None, ADD)
                y = seed[:].bitcast(F32)
                for it in range(2):
                    a_ = spool.tile([128, 1], F32, tag=f"nra{it}", name=f"nra{it}")
                    nc.vector.tensor_tensor(out=a_[:], in0=y, in1=y, op=MULT)
                    b_ = spool.tile([128, 1], F32, tag=f"nrb{it}", name=f"nrb{it}")
                    nc.vector.tensor_tensor(out=b_[:], in0=a_[:], in1=var, op=MULT)
                    c_ = spool.tile([128, 1], F32, tag=f"nrc{it}", name=f"nrc{it}")
                    nc.vector.tensor_scalar(c_[:], b_[:], -0.5, 1.5, MULT, ADD)
                    y2 = spool.tile([128, 1], F32, tag=f"nry{it}", name=f"nry{it}")
                    nc.vector.tensor_tensor(out=y2[:], in0=y, in1=c_[:], op=MULT)
                    y = y2[:]
                mr = spool.tile([128, 1], F32, tag="mr")
                nc.vector.tensor_tensor(out=mr[:], in0=mean, in1=y, op=MULT)
                negmr = spool.tile([128, 1], F32, tag="negmr")
                nc.vector.tensor_scalar(negmr[:], mr[:], -1.0, None, MULT)
                onrm = lpool.tile([128, D], F32, tag="onrm")
                nc.vector.tensor_scalar(onrm[:], osb[:], y, negmr[:], MULT, ADD)
                # lnw/lnb products on the Pool (SBUF-only engine)
                ow = lpool.tile([128, D], F32, tag="ow")
                nc.gpsimd.tensor_tensor(out=ow[:], in0=onrm[:], in1=lnw[:], op=MULT)
                ofin = lpool.tile([128, D], F32, tag="ofin")
                nc.gpsimd.tensor_tensor(out=ofin[:], in0=ow[:], in1=lnb[:], op=ADD)
                nc.scalar.dma_start(outd[128 * w : 128 * (w + 1), :], ofin[:])

            # ---- top-level schedule
            emit_A(0)
            emit_A(1)
            emit_window(0, {1: lambda: emit_A(2), 3: lambda: emit_A(3)})
            emit_norm_a2a(0)
            emit_window(
                1,
                {
                    0: lambda: emit_A(4),
                    2: lambda: emit_A(5),
                    4: lambda: emit_A(6),
                    6: lambda: emit_A(7),
                },
            )
            emit_norm_a2a(1)
            c_osb = {}

            def c_mm(w):
                c_osb[w] = emit_C_mm(w)

            emit_window(2, {1: lambda: c_mm(0), 4: lambda: emit_C_ln(0, c_osb[0])})
            emit_norm_a2a(2)
            emit_window(3, {1: lambda: c_mm(1), 4: lambda: emit_C_ln(1, c_osb[1])})
            emit_norm_a2a(3)
            c_mm(2)
            emit_C_ln(2, c_osb[2])
            emit_warm(14, "warmtail")
            c_mm(3)
            emit_C_ln(3, c_osb[3])
            if _dump:
                nc.sync.dma_start(q8o[:], qS8[0:64].rearrange("p a b -> p (a b)"))
                for h in range(HPC):
                    nc.sync.dma_start(
                        k8o[32 * h : 32 * (h + 1), :],
                        kz[h][32 * h : 32 * (h + 1)].rearrange("p a b -> p (a b)"),
                    )
                nc.sync.dma_start(v8o[:], vaug[:].rearrange("p a b c d -> p (a b c d)"))

    nc.finalize()
    return nc


# feature permutation: partition p of the projected q/k carries original
# head-feature fperm[p]; head h's d-half i lives at partitions [32h,32h+32)
# free-slot i, which is partition block 64i + 32h of the projection output
def _fperm():
    p = np.empty(128, np.int64)
    for h in range(HPC):
        for dd in range(32):
            p[32 * h + dd] = 64 * h + dd            # d_lo at blocks 0/1
            p[64 + 32 * h + dd] = 64 * h + 32 + dd  # d_hi at blocks 2/3
    return p


def _to_fp8(a):
    return np.ascontiguousarray(a.astype(ml_dtypes.float8_e4m3))


def _prep_in_maps(hidden_states, cos, sin, Wq, bq, Wk, bk, Wv, bv, Wo, bo, ln_w, ln_b):
    hs = np.ascontiguousarray(np.asarray(hidden_states, np.float32).reshape(NSEQ, D))
    cos = np.asarray(cos, np.float32)
    sin = np.asarray(sin, np.float32)
    Wq = np.asarray(Wq, np.float32)
    bq = np.asarray(bq, np.float32)
    Wk = np.asarray(Wk, np.float32)
    bk = np.asarray(bk, np.float32)
    Wv = np.asarray(Wv, np.float32)
    bv = np.asarray(bv, np.float32)
    Wo = np.asarray(Wo, np.float32)
    bo = np.asarray(bo, np.float32)
    ln_w = np.asarray(ln_w, np.float32)
    ln_b = np.asarray(ln_b, np.float32)

    fperm = _fperm()
    xT8 = _to_fp8(hs.T)
    cosT = cos.T
    sinT = sin.T
    # row p carries cos/sin of d = fperm[p] % 64; sn2 folds the rotate-half
    # sign (rows 0:64 are the d_lo outputs, sourced from -q[d_hi])
    cs2 = np.ascontiguousarray(cosT[fperm % 64].astype(np.float16))
    sgn = np.where(np.arange(128) < 64, -1.0, 1.0)[:, None].astype(np.float32)
    sn2 = np.ascontiguousarray((sgn * sinT[fperm % 64]).astype(np.float16))
    lnw_t = np.ascontiguousarray(np.tile(ln_w[None, :], (128, 1)))
    lnb_t = np.ascontiguousarray(np.tile(ln_b[None, :], (128, 1)))
    # Wo scaled x4 into fp8's normal range, laid out for DoubleRow rhs
    wo = 4.0 * Wo.T                                   # [din, dout]
    wo8 = wo.reshape(4, 2, 128, 2, 512)               # [cp, i, p, n, dout]
    wo8 = wo8.transpose(2, 3, 0, 1, 4)                # [p, n, cp, i, dout]
    wo8 = _to_fp8(wo8.reshape(128, 2 * 4 * 2 * 512))

    def w8_layout(WT_cols):
        # WT_cols: [D, 128] = W.T slice for this core's features (scaled)
        # -> [128 part, (4 cpair, 2 half, 128 feat)] fp8
        a = WT_cols.reshape(4, 2, 128, 128)          # [cp, i, p, f]
        a = a.transpose(2, 0, 1, 3)                   # [p, cp, i, f]
        return _to_fp8(a.reshape(128, D))

    # x16: matches the 4*ctx fp8 ctile times the x4 fp8 Wo; LayerNorm is
    # scale-invariant so the output is unchanged
    resid_full = WSCALE * (hs + bo[None, :])

    in_maps = []
    for c in range(NC):
        fs = np.arange(F * c, F * (c + 1))
        fs_p = fs[0] + fperm                          # permuted q/k features
        rows = np.concatenate(
            [resid_full[1024 * w + 128 * c : 1024 * w + 128 * (c + 1)] for w in range(NW)],
            axis=0,
        )
        in_maps.append(
            {
                "xT": xT8,
                "wq": w8_layout(WSCALE * Wq.T[:, fs_p]),
                "wk": w8_layout(WSCALE * Wk.T[:, fs_p]),
                "wv": w8_layout(WSCALE * Wv.T[:, fs]),
                "bq": np.ascontiguousarray(WSCALE * bq[fs_p, None]),
                "bk": np.ascontiguousarray(WSCALE * bk[fs_p, None]),
                "bv": np.ascontiguousarray(WSCALE * bv[fs, None]),
                "wo8": wo8,
                "cs2": cs2,
                "sn2": sn2,
                "resid": np.ascontiguousarray(rows),
                "lnw": lnw_t,
                "lnb": lnb_t,
            }
        )
    return in_maps


def _unshard(results):
    out = np.empty((NSEQ, D), np.float32)
    for c in range(NC):
        res = results[c]["out"]
        for w in range(NW):
            out[1024 * w + 128 * c : 1024 * w + 128 * (c + 1)] = res[
                128 * w : 128 * (w + 1)
            ]
    return out.reshape(B, S, D)


def kernel(hidden_states, cos, sin, Wq, bq, Wk, bk, Wv, bv, Wo, bo, ln_w, ln_b):
    global LAST_RESULTS
    in_maps = _prep_in_maps(
        hidden_states, cos, sin, Wq, bq, Wk, bk, Wv, bv, Wo, bo, ln_w, ln_b
    )
    nc = _build()
    LAST_RESULTS = run_bass_kernel_spmd(nc, in_maps, core_ids=list(range(NC)))
    out = np.empty((NSEQ, D), np.float32)
    for c in range(NC):
        res = LAST_RESULTS.results[c]["out"]
        for w in range(NW):
            out[1024 * w + 128 * c : 1024 * w + 128 * (c + 1)] = res[
                128 * w : 128 * (w + 1)
            ]
    return out.reshape(B, S, D)
